# revision 3
# baseline (speedup 1.0000x reference)
"""LinearRecurrentUnitCell Bass/Tile kernel for 8 Trainium2 NeuronCores.

Computation (per reference):
    lam = exp(-exp(nu)) * exp(i*theta)                  # [H] complex
    U   = gamma * (inputs @ (B_re + i B_im))            # [B,H] complex
    x'  = (state_re + i state_im) * lam + U             # [B,H] complex
    y   = Re(x' @ (C_re + i C_im)) + D * inputs         # [B,N]
    returns (y, Re(x'), Im(x'))

Sharding: data-parallel over batch (8192 -> 8 x 1024). Parameters replicated.

Device layout strategy ("feature-major"): all activations are kept
transposed (feature dim on partitions) so every matmul uses operands
exactly as stored -- no on-device transposes:
  phase 1:  UT[h,b]  = sum_n B[n,h] * uT[n,b]      (lhsT=B tile, rhs=uT)
  elemwise: xT = lam (.) sT + gamma (.) UT          (per-partition scalars)
  phase 2:  y[b,n]   = sum_h xT[h,b] * C[h,n]       (lhsT=xT tile, rhs=C tile)
Host transposes inputs/states in and xT out (numpy; not on-HW time).
"""

import os

import numpy as np

import concourse.bass as bass
import concourse.tile as tile
from concourse import bacc, mybir
from concourse.bass_utils import run_bass_kernel_spmd

P = 128            # partitions
N = 1024           # n_rec
H = 2048           # d_hidden
BC = 1024          # batch per core
NCORES = 8
HO = H // P        # 16 h-tiles
KO = N // P        # 8 k-tiles (phase 1 contraction)
BT = BC // P       # 8 batch tiles (phase 2 output partition)
NF = 512           # matmul moving free dim (fp32 max)

F32 = mybir.dt.float32
F32R = mybir.dt.float32r

# matmul dtype: "f32r" = relaxed-precision fp32 at full PE rate,
# "f32" = exact fp32 at 1/4 PE rate.
MM_MODE = os.environ.get("LRU_MM_DT", "f32r")
MDT = F32R if MM_MODE == "f32r" else F32


_BUILT = {}


def _build():
    if "nc" in _BUILT:
        return _BUILT["nc"]

    nc = bacc.Bacc("TRN2", target_bir_lowering=False, debug=False)

    # --- DRAM I/O (per-core shapes) ---
    uT_d = nc.dram_tensor("uT", [N, BC], MDT, kind="ExternalInput").ap()
    ubm_d = nc.dram_tensor("u_bm", [BC, N], F32, kind="ExternalInput").ap()
    sre_d = nc.dram_tensor("sT_re", [H, BC], F32, kind="ExternalInput").ap()
    sim_d = nc.dram_tensor("sT_im", [H, BC], F32, kind="ExternalInput").ap()
    # B pre-tiled on host: [HO, P(n_inner), KO, P(h_inner)]
    bre_d = nc.dram_tensor("B_pre_re", [HO, P, KO, P], MDT, kind="ExternalInput").ap()
    bim_d = nc.dram_tensor("B_pre_im", [HO, P, KO, P], MDT, kind="ExternalInput").ap()
    # C pre-tiled on host: [2(n_half), HO, P(h_inner), NF]; C_imn = -C_im
    cre_d = nc.dram_tensor("C_pre_re", [2, HO, P, NF], MDT, kind="ExternalInput").ap()
    cim_d = nc.dram_tensor("C_pre_imn", [2, HO, P, NF], MDT, kind="ExternalInput").ap()
    # per-feature vectors, host-shaped [P, HO]
    lre_d = nc.dram_tensor("lam_re", [P, HO], F32, kind="ExternalInput").ap()
    lim_d = nc.dram_tensor("lam_im", [P, HO], F32, kind="ExternalInput").ap()
    limn_d = nc.dram_tensor("lam_imn", [P, HO], F32, kind="ExternalInput").ap()
    gam_d = nc.dram_tensor("gamma_v", [P, HO], F32, kind="ExternalInput").ap()
    dbc_d = nc.dram_tensor("D_bcast", [P, N], F32, kind="ExternalInput").ap()

    xre_d = nc.dram_tensor("xT_re", [H, BC], MDT, kind="ExternalOutput").ap()
    xim_d = nc.dram_tensor("xT_im", [H, BC], MDT, kind="ExternalOutput").ap()
    y_d = nc.dram_tensor("y", [BC, N], F32, kind="ExternalOutput").ap()

    MULT = mybir.AluOpType.mult
    ADD = mybir.AluOpType.add
    COPY = mybir.ActivationFunctionType.Copy

    with tile.TileContext(nc) as tc:
        with tc.tile_pool(name="const", bufs=1) as const:
            lre = const.tile([P, HO], F32, tag="lre")
            nc.sync.dma_start(lre[:], lre_d)
            lim = const.tile([P, HO], F32, tag="lim")
            nc.sync.dma_start(lim[:], lim_d)
            limn = const.tile([P, HO], F32, tag="limn")
            nc.sync.dma_start(limn[:], limn_d)
            gam = const.tile([P, HO], F32, tag="gam")
            nc.sync.dma_start(gam[:], gam_d)
            dbc = const.tile([P, N], F32, tag="dbc")
            nc.sync.dma_start(dbc[:], dbc_d)

            with tc.tile_pool(name="xres", bufs=1) as xres:
                xre_sb = xres.tile([P, HO, BC], MDT, tag="xre")
                xim_sb = xres.tile([P, HO, BC], MDT, tag="xim")

                # ---------------- phase 1 ----------------
                with (
                    tc.tile_pool(name="upool", bufs=1) as upool,
                    tc.tile_pool(name="bw", bufs=2) as bw,
                    tc.tile_pool(name="st", bufs=2) as st,
                    tc.tile_pool(name="ps1", bufs=2, space="PSUM") as ps1,
                ):
                    uT_sb = upool.tile([P, KO, BC], MDT, tag="uT")
                    uT_r = uT_d.rearrange("(ko p) b -> p ko b", p=P)
                    for ko in range(KO):
                        nc.sync.dma_start(uT_sb[:, ko, :], uT_r[:, ko, :])

                    for ho in range(HO):
                        bre_t = bw.tile([P, KO, P], MDT, tag="bre")
                        nc.sync.dma_start(bre_t[:], bre_d[ho])
                        bim_t = bw.tile([P, KO, P], MDT, tag="bim")
                        nc.sync.dma_start(bim_t[:], bim_d[ho])
                        sre_t = st.tile([P, BC], F32, tag="sre")
                        nc.sync.dma_start(sre_t[:], sre_d[ho * P:(ho + 1) * P, :])
                        sim_t = st.tile([P, BC], F32, tag="sim")
                        nc.sync.dma_start(sim_t[:], sim_d[ho * P:(ho + 1) * P, :])

                        ps_u = {}
                        for comp, b_t in (("re", bre_t), ("im", bim_t)):
                            for f in range(2):
                                ps = ps1.tile([P, NF], F32, tag=f"psu_{comp}_{f}")
                                for ko in range(KO):
                                    nc.tensor.matmul(
                                        ps[:],
                                        lhsT=b_t[:, ko, :],
                                        rhs=uT_sb[:, ko, f * NF:(f + 1) * NF],
                                        start=(ko == 0),
                                        stop=(ko == KO - 1),
                                    )
                                ps_u[(comp, f)] = ps

                        lre_s = lre[:, ho:ho + 1]
                        lim_s = lim[:, ho:ho + 1]
                        limn_s = limn[:, ho:ho + 1]
                        gam_s = gam[:, ho:ho + 1]

                        # x_re = s_re*l_re - s_im*l_im + gam*U_re
                        xre_o = xre_sb[:, ho, :]
                        nc.scalar.activation(xre_o, sre_t[:], COPY, scale=lre_s)
                        nc.vector.scalar_tensor_tensor(
                            xre_o, sim_t[:], limn_s, xre_o, op0=MULT, op1=ADD)
                        # x_im = s_re*l_im + s_im*l_re + gam*U_im
                        xim_o = xim_sb[:, ho, :]
                        nc.scalar.activation(xim_o, sim_t[:], COPY, scale=lre_s)
                        nc.vector.scalar_tensor_tensor(
                            xim_o, sre_t[:], lim_s, xim_o, op0=MULT, op1=ADD)
                        for f in range(2):
                            fs = slice(f * NF, (f + 1) * NF)
                            nc.vector.scalar_tensor_tensor(
                                xre_o[:, fs], ps_u[("re", f)][:], gam_s,
                                xre_o[:, fs], op0=MULT, op1=ADD)
                            nc.vector.scalar_tensor_tensor(
                                xim_o[:, fs], ps_u[("im", f)][:], gam_s,
                                xim_o[:, fs], op0=MULT, op1=ADD)

                # ---------------- phase 2 ----------------
                with (
                    tc.tile_pool(name="cw", bufs=3) as cw,
                    tc.tile_pool(name="ev", bufs=2) as ev,
                    tc.tile_pool(name="ps2", bufs=1, space="PSUM") as ps2,
                ):
                    for nf in range(2):
                        nfs = slice(nf * NF, (nf + 1) * NF)
                        ps_y = [ps2.tile([P, NF], F32, tag=f"psy_{bt}",
                                         name=f"psy_{nf}_{bt}")
                                for bt in range(BT)]
                        for ho in range(HO):
                            cre_t = cw.tile([P, NF], MDT, tag="cre")
                            nc.sync.dma_start(cre_t[:], cre_d[nf, ho])
                            cim_t = cw.tile([P, NF], MDT, tag="cim")
                            nc.sync.dma_start(cim_t[:], cim_d[nf, ho])
                            for bt in range(BT):
                                bs = slice(bt * P, (bt + 1) * P)
                                nc.tensor.matmul(
                                    ps_y[bt][:],
                                    lhsT=xre_sb[:, ho, bs],
                                    rhs=cre_t[:],
                                    start=(ho == 0),
                                    stop=False,
                                )
                                nc.tensor.matmul(
                                    ps_y[bt][:],
                                    lhsT=xim_sb[:, ho, bs],
                                    rhs=cim_t[:],
                                    start=False,
                                    stop=(ho == HO - 1),
                                )
                            if nf == 0:
                                # stream x out under phase-2 compute
                                nc.sync.dma_start(
                                    xre_d[ho * P:(ho + 1) * P, :], xre_sb[:, ho, :])
                                nc.sync.dma_start(
                                    xim_d[ho * P:(ho + 1) * P, :], xim_sb[:, ho, :])
                        for bt in range(BT):
                            bs = slice(bt * P, (bt + 1) * P)
                            u_t = ev.tile([P, NF], F32, tag="ubm")
                            nc.sync.dma_start(u_t[:], ubm_d[bs, nfs])
                            du = ev.tile([P, NF], F32, tag="du")
                            nc.vector.tensor_tensor(
                                du[:], u_t[:], dbc[:, nfs], MULT)
                            y_t = ev.tile([P, NF], F32, tag="yt")
                            nc.vector.tensor_tensor(
                                y_t[:], ps_y[bt][:], du[:], ADD)
                            nc.sync.dma_start(y_d[bs, nfs], y_t[:])

    nc.compile()
    _BUILT["nc"] = nc
    return nc


def kernel(inputs, state_re, state_im, B_re, B_im, C_re, C_im, D, nu, theta,
           gamma):
    nc = _build()

    f32 = np.float32
    # host-side parameter prep (fp64 for transcendentals, cast to f32)
    nu64 = nu.astype(np.float64)
    lam_mod = np.exp(-np.exp(nu64))
    lam_re = (lam_mod * np.cos(theta.astype(np.float64))).astype(f32)
    lam_im = (lam_mod * np.sin(theta.astype(np.float64))).astype(f32)

    def vec_ph(v):  # [H] -> [P, HO] with v[ho*P + p] at [p, ho]
        return np.ascontiguousarray(v.reshape(HO, P).T)

    B_pre_re = np.ascontiguousarray(
        B_re.reshape(KO, P, HO, P).transpose(2, 1, 0, 3))
    B_pre_im = np.ascontiguousarray(
        B_im.reshape(KO, P, HO, P).transpose(2, 1, 0, 3))
    C_pre_re = np.ascontiguousarray(
        C_re.reshape(HO, P, 2, NF).transpose(2, 0, 1, 3))
    C_pre_imn = np.ascontiguousarray(
        (-C_im).reshape(HO, P, 2, NF).transpose(2, 0, 1, 3))

    shared = {
        "B_pre_re": B_pre_re,
        "B_pre_im": B_pre_im,
        "C_pre_re": C_pre_re,
        "C_pre_imn": C_pre_imn,
        "lam_re": vec_ph(lam_re),
        "lam_im": vec_ph(lam_im),
        "lam_imn": vec_ph(-lam_im),
        "gamma_v": vec_ph(gamma.astype(f32)),
        "D_bcast": np.ascontiguousarray(np.broadcast_to(D.astype(f32), (P, N))),
    }

    in_maps = []
    for c in range(NCORES):
        rows = slice(c * BC, (c + 1) * BC)
        in_maps.append({
            **shared,
            "uT": np.ascontiguousarray(inputs[rows].T),
            "u_bm": np.ascontiguousarray(inputs[rows]),
            "sT_re": np.ascontiguousarray(state_re[rows].T),
            "sT_im": np.ascontiguousarray(state_im[rows].T),
        })

    res = run_bass_kernel_spmd(nc, in_maps, core_ids=list(range(NCORES)))

    y = np.empty((NCORES * BC, N), f32)
    x_re = np.empty((NCORES * BC, H), f32)
    x_im = np.empty((NCORES * BC, H), f32)
    for c in range(NCORES):
        rows = slice(c * BC, (c + 1) * BC)
        r = res.results[c]
        y[rows] = r["y"]
        x_re[rows] = r["xT_re"].T
        x_im[rows] = r["xT_im"].T
    return y, x_re, x_im


# revision 4
# speedup vs baseline: 114.5522x; 114.5522x over previous
"""LinearRecurrentUnitCell Bass/Tile kernel for 8 Trainium2 NeuronCores.

Computation (per reference):
    lam = exp(-exp(nu)) * exp(i*theta)                  # [H] complex
    U   = gamma * (inputs @ (B_re + i B_im))            # [B,H] complex
    x'  = (state_re + i state_im) * lam + U             # [B,H] complex
    y   = Re(x' @ (C_re + i C_im)) + D * inputs         # [B,N]
    returns (y, Re(x'), Im(x'))

Sharding: data-parallel over batch (8192 -> 8 x 1024). Parameters replicated.

Device layout strategy ("feature-major"): all activations are kept
transposed (feature dim on partitions) so every matmul uses operands
exactly as stored -- no on-device transposes:
  phase 1:  UT[h,b]  = sum_n B[n,h] * uT[n,b]      (lhsT=B tile, rhs=uT)
  elemwise: xT = lam (.) sT + gamma (.) UT          (per-partition scalars)
  phase 2:  y[b,n]   = sum_h xT[h,b] * C[h,n]       (lhsT=xT tile, rhs=C tile)
Host transposes inputs/states in and xT out (numpy; not on-HW time).
"""

import os

import numpy as np

import concourse.bass as bass
import concourse.tile as tile
from concourse import bacc, mybir
from concourse.bass_utils import run_bass_kernel_spmd

P = 128            # partitions
N = 1024           # n_rec
H = 2048           # d_hidden
BC = 1024          # batch per core
NCORES = 8
HO = H // P        # 16 h-tiles
KO = N // P        # 8 k-tiles (phase 1 contraction)
BT = BC // P       # 8 batch tiles (phase 2 output partition)
NF = 512           # matmul moving free dim (fp32 max)

F32 = mybir.dt.float32
F32R = mybir.dt.float32r

# matmul dtype: "f32r" = relaxed-precision fp32 at full PE rate,
# "f32" = exact fp32 at 1/4 PE rate.
MM_MODE = os.environ.get("LRU_MM_DT", "f32r")
MDT = F32R if MM_MODE == "f32r" else F32


_BUILT = {}


def _build(repeat=1):
    if repeat in _BUILT:
        return _BUILT[repeat]

    nc = bacc.Bacc("TRN2", target_bir_lowering=False, debug=False)

    # --- DRAM I/O (per-core shapes) ---
    uT_d = nc.dram_tensor("uT", [N, BC], MDT, kind="ExternalInput").ap()
    ubm_d = nc.dram_tensor("u_bm", [BC, N], F32, kind="ExternalInput").ap()
    sre_d = nc.dram_tensor("sT_re", [H, BC], F32, kind="ExternalInput").ap()
    sim_d = nc.dram_tensor("sT_im", [H, BC], F32, kind="ExternalInput").ap()
    # B pre-tiled on host: [HO, P(n_inner), KO, P(h_inner)]
    bre_d = nc.dram_tensor("B_pre_re", [HO, P, KO, P], MDT, kind="ExternalInput").ap()
    bim_d = nc.dram_tensor("B_pre_im", [HO, P, KO, P], MDT, kind="ExternalInput").ap()
    # C pre-tiled on host: [2(n_half), HO, P(h_inner), NF]; C_imn = -C_im
    cre_d = nc.dram_tensor("C_pre_re", [2, HO, P, NF], MDT, kind="ExternalInput").ap()
    cim_d = nc.dram_tensor("C_pre_imn", [2, HO, P, NF], MDT, kind="ExternalInput").ap()
    # per-feature vectors, host-shaped [P, HO]
    lre_d = nc.dram_tensor("lam_re", [P, HO], F32, kind="ExternalInput").ap()
    lim_d = nc.dram_tensor("lam_im", [P, HO], F32, kind="ExternalInput").ap()
    limn_d = nc.dram_tensor("lam_imn", [P, HO], F32, kind="ExternalInput").ap()
    gam_d = nc.dram_tensor("gamma_v", [P, HO], F32, kind="ExternalInput").ap()
    dbc_d = nc.dram_tensor("D_bcast", [P, N], F32, kind="ExternalInput").ap()

    xre_d = nc.dram_tensor("xT_re", [H, BC], MDT, kind="ExternalOutput").ap()
    xim_d = nc.dram_tensor("xT_im", [H, BC], MDT, kind="ExternalOutput").ap()
    y_d = nc.dram_tensor("y", [BC, N], F32, kind="ExternalOutput").ap()

    MULT = mybir.AluOpType.mult
    ADD = mybir.AluOpType.add
    COPY = mybir.ActivationFunctionType.Copy

    with tile.TileContext(nc) as tc:
      for _rep in range(repeat):
        with tc.tile_pool(name="const", bufs=1) as const:
            lre = const.tile([P, HO], F32, tag="lre")
            nc.sync.dma_start(lre[:], lre_d)
            lim = const.tile([P, HO], F32, tag="lim")
            nc.sync.dma_start(lim[:], lim_d)
            limn = const.tile([P, HO], F32, tag="limn")
            nc.sync.dma_start(limn[:], limn_d)
            gam = const.tile([P, HO], F32, tag="gam")
            nc.sync.dma_start(gam[:], gam_d)
            dbc = const.tile([P, N], F32, tag="dbc")
            nc.sync.dma_start(dbc[:], dbc_d)

            with tc.tile_pool(name="xres", bufs=1) as xres:
                xre_sb = xres.tile([P, HO, BC], MDT, tag="xre")
                xim_sb = xres.tile([P, HO, BC], MDT, tag="xim")

                # ---------------- phase 1 ----------------
                with (
                    tc.tile_pool(name="upool", bufs=1) as upool,
                    tc.tile_pool(name="bw", bufs=2) as bw,
                    tc.tile_pool(name="st", bufs=2) as st,
                    tc.tile_pool(name="ps1", bufs=2, space="PSUM") as ps1,
                ):
                    uT_sb = upool.tile([P, KO, BC], MDT, tag="uT")
                    uT_r = uT_d.rearrange("(ko p) b -> p ko b", p=P)
                    for ko in range(KO):
                        nc.sync.dma_start(uT_sb[:, ko, :], uT_r[:, ko, :])

                    for ho in range(HO):
                        bre_t = bw.tile([P, KO, P], MDT, tag="bre")
                        nc.sync.dma_start(bre_t[:], bre_d[ho])
                        bim_t = bw.tile([P, KO, P], MDT, tag="bim")
                        nc.sync.dma_start(bim_t[:], bim_d[ho])
                        sre_t = st.tile([P, BC], F32, tag="sre")
                        nc.sync.dma_start(sre_t[:], sre_d[ho * P:(ho + 1) * P, :])
                        sim_t = st.tile([P, BC], F32, tag="sim")
                        nc.sync.dma_start(sim_t[:], sim_d[ho * P:(ho + 1) * P, :])

                        ps_u = {}
                        for comp, b_t in (("re", bre_t), ("im", bim_t)):
                            for f in range(2):
                                ps = ps1.tile([P, NF], F32, tag=f"psu_{comp}_{f}")
                                for ko in range(KO):
                                    nc.tensor.matmul(
                                        ps[:],
                                        lhsT=b_t[:, ko, :],
                                        rhs=uT_sb[:, ko, f * NF:(f + 1) * NF],
                                        start=(ko == 0),
                                        stop=(ko == KO - 1),
                                    )
                                ps_u[(comp, f)] = ps

                        lre_s = lre[:, ho:ho + 1]
                        lim_s = lim[:, ho:ho + 1]
                        limn_s = limn[:, ho:ho + 1]
                        gam_s = gam[:, ho:ho + 1]

                        # x_re = s_re*l_re - s_im*l_im + gam*U_re
                        xre_o = xre_sb[:, ho, :]
                        nc.scalar.activation(xre_o, sre_t[:], COPY, scale=lre_s)
                        nc.vector.scalar_tensor_tensor(
                            xre_o, sim_t[:], limn_s, xre_o, op0=MULT, op1=ADD)
                        # x_im = s_re*l_im + s_im*l_re + gam*U_im
                        xim_o = xim_sb[:, ho, :]
                        nc.scalar.activation(xim_o, sim_t[:], COPY, scale=lre_s)
                        nc.vector.scalar_tensor_tensor(
                            xim_o, sre_t[:], lim_s, xim_o, op0=MULT, op1=ADD)
                        for f in range(2):
                            fs = slice(f * NF, (f + 1) * NF)
                            nc.vector.scalar_tensor_tensor(
                                xre_o[:, fs], ps_u[("re", f)][:], gam_s,
                                xre_o[:, fs], op0=MULT, op1=ADD)
                            nc.vector.scalar_tensor_tensor(
                                xim_o[:, fs], ps_u[("im", f)][:], gam_s,
                                xim_o[:, fs], op0=MULT, op1=ADD)

                # ---------------- phase 2 ----------------
                with (
                    tc.tile_pool(name="cw", bufs=3) as cw,
                    tc.tile_pool(name="ev", bufs=2) as ev,
                    tc.tile_pool(name="ps2", bufs=1, space="PSUM") as ps2,
                ):
                    for nf in range(2):
                        nfs = slice(nf * NF, (nf + 1) * NF)
                        ps_y = [ps2.tile([P, NF], F32, tag=f"psy_{bt}",
                                         name=f"psy_{nf}_{bt}")
                                for bt in range(BT)]
                        for ho in range(HO):
                            cre_t = cw.tile([P, NF], MDT, tag="cre")
                            nc.sync.dma_start(cre_t[:], cre_d[nf, ho])
                            cim_t = cw.tile([P, NF], MDT, tag="cim")
                            nc.sync.dma_start(cim_t[:], cim_d[nf, ho])
                            for bt in range(BT):
                                bs = slice(bt * P, (bt + 1) * P)
                                nc.tensor.matmul(
                                    ps_y[bt][:],
                                    lhsT=xre_sb[:, ho, bs],
                                    rhs=cre_t[:],
                                    start=(ho == 0),
                                    stop=False,
                                )
                                nc.tensor.matmul(
                                    ps_y[bt][:],
                                    lhsT=xim_sb[:, ho, bs],
                                    rhs=cim_t[:],
                                    start=False,
                                    stop=(ho == HO - 1),
                                )
                            if nf == 0:
                                # stream x out under phase-2 compute
                                nc.sync.dma_start(
                                    xre_d[ho * P:(ho + 1) * P, :], xre_sb[:, ho, :])
                                nc.sync.dma_start(
                                    xim_d[ho * P:(ho + 1) * P, :], xim_sb[:, ho, :])
                        for bt in range(BT):
                            bs = slice(bt * P, (bt + 1) * P)
                            u_t = ev.tile([P, NF], F32, tag="ubm")
                            nc.sync.dma_start(u_t[:], ubm_d[bs, nfs])
                            du = ev.tile([P, NF], F32, tag="du")
                            nc.vector.tensor_tensor(
                                du[:], u_t[:], dbc[:, nfs], MULT)
                            y_t = ev.tile([P, NF], F32, tag="yt")
                            nc.vector.tensor_tensor(
                                y_t[:], ps_y[bt][:], du[:], ADD)
                            nc.sync.dma_start(y_d[bs, nfs], y_t[:])

    nc.compile()
    _BUILT[repeat] = nc
    return nc


def kernel(inputs, state_re, state_im, B_re, B_im, C_re, C_im, D, nu, theta,
           gamma):
    nc = _build()

    f32 = np.float32
    # host-side parameter prep (fp64 for transcendentals, cast to f32)
    nu64 = nu.astype(np.float64)
    lam_mod = np.exp(-np.exp(nu64))
    lam_re = (lam_mod * np.cos(theta.astype(np.float64))).astype(f32)
    lam_im = (lam_mod * np.sin(theta.astype(np.float64))).astype(f32)

    def vec_ph(v):  # [H] -> [P, HO] with v[ho*P + p] at [p, ho]
        return np.ascontiguousarray(v.reshape(HO, P).T)

    B_pre_re = np.ascontiguousarray(
        B_re.reshape(KO, P, HO, P).transpose(2, 1, 0, 3))
    B_pre_im = np.ascontiguousarray(
        B_im.reshape(KO, P, HO, P).transpose(2, 1, 0, 3))
    C_pre_re = np.ascontiguousarray(
        C_re.reshape(HO, P, 2, NF).transpose(2, 0, 1, 3))
    C_pre_imn = np.ascontiguousarray(
        (-C_im).reshape(HO, P, 2, NF).transpose(2, 0, 1, 3))

    shared = {
        "B_pre_re": B_pre_re,
        "B_pre_im": B_pre_im,
        "C_pre_re": C_pre_re,
        "C_pre_imn": C_pre_imn,
        "lam_re": vec_ph(lam_re),
        "lam_im": vec_ph(lam_im),
        "lam_imn": vec_ph(-lam_im),
        "gamma_v": vec_ph(gamma.astype(f32)),
        "D_bcast": np.ascontiguousarray(np.broadcast_to(D.astype(f32), (P, N))),
    }

    in_maps = []
    for c in range(NCORES):
        rows = slice(c * BC, (c + 1) * BC)
        in_maps.append({
            **shared,
            "uT": np.ascontiguousarray(inputs[rows].T),
            "u_bm": np.ascontiguousarray(inputs[rows]),
            "sT_re": np.ascontiguousarray(state_re[rows].T),
            "sT_im": np.ascontiguousarray(state_im[rows].T),
        })

    res = run_bass_kernel_spmd(nc, in_maps, core_ids=list(range(NCORES)))

    y = np.empty((NCORES * BC, N), f32)
    x_re = np.empty((NCORES * BC, H), f32)
    x_im = np.empty((NCORES * BC, H), f32)
    for c in range(NCORES):
        rows = slice(c * BC, (c + 1) * BC)
        r = res.results[c]
        y[rows] = r["y"]
        x_re[rows] = r["xT_re"].T
        x_im[rows] = r["xT_im"].T
    return y, x_re, x_im


# revision 7
# speedup vs baseline: 120.3830x; 1.0509x over previous
"""LinearRecurrentUnitCell Bass/Tile kernel for 8 Trainium2 NeuronCores.

Computation (per reference):
    lam = exp(-exp(nu)) * exp(i*theta)                  # [H] complex
    U   = gamma * (inputs @ (B_re + i B_im))            # [B,H] complex
    x'  = (state_re + i state_im) * lam + U             # [B,H] complex
    y   = Re(x' @ (C_re + i C_im)) + D * inputs         # [B,N]
    returns (y, Re(x'), Im(x'))

Sharding: data-parallel over batch (8192 -> 8 x 1024). Parameters replicated.

Device layout strategy ("feature-major"): all activations are kept
transposed (feature dim on partitions) so every matmul uses operands
exactly as stored -- no on-device transposes:
  phase 1:  UT[h,b]  = sum_n B[n,h] * uT[n,b]      (lhsT=B tile, rhs=uT)
  elemwise: xT = lam (.) sT + gamma (.) UT          (per-partition scalars)
  phase 2:  y[b,n]   = sum_h xT[h,b] * C[h,n]       (lhsT=xT tile, rhs=C tile)
Host transposes inputs/states in and xT out (numpy; not on-HW time).
"""

import os

import numpy as np

import concourse.bass as bass
import concourse.tile as tile
from concourse import bacc, mybir
from concourse.bass_utils import run_bass_kernel_spmd

P = 128            # partitions
N = 1024           # n_rec
H = 2048           # d_hidden
BC = 1024          # batch per core
NCORES = 8
HO = H // P        # 16 h-tiles
KO = N // P        # 8 k-tiles (phase 1 contraction)
BT = BC // P       # 8 batch tiles (phase 2 output partition)
NF = 512           # matmul moving free dim (fp32 max)

F32 = mybir.dt.float32
F32R = mybir.dt.float32r

# matmul dtype: "f32r" = relaxed-precision fp32 at full PE rate,
# "f32" = exact fp32 at 1/4 PE rate.
MM_MODE = os.environ.get("LRU_MM_DT", "f32r")
MDT = F32R if MM_MODE == "f32r" else F32


_BUILT = {}


def _build(repeat=1, variant="full"):
    key = (repeat, variant)
    if key in _BUILT:
        return _BUILT[key]
    mm_only = variant == "mm_only"
    no_mm = variant == "no_mm"

    nc = bacc.Bacc("TRN2", target_bir_lowering=False, debug=False)

    # --- DRAM I/O (per-core shapes) ---
    uT_d = nc.dram_tensor("uT", [N, BC], MDT, kind="ExternalInput").ap()
    ubm_d = nc.dram_tensor("u_bm", [BC, N], F32, kind="ExternalInput").ap()
    sre_d = nc.dram_tensor("sT_re", [H, BC], F32, kind="ExternalInput").ap()
    sim_d = nc.dram_tensor("sT_im", [H, BC], F32, kind="ExternalInput").ap()
    # B pre-tiled on host: [HO, P(n_inner), KO, P(h_inner)]
    bre_d = nc.dram_tensor("B_pre_re", [HO, P, KO, P], MDT, kind="ExternalInput").ap()
    bim_d = nc.dram_tensor("B_pre_im", [HO, P, KO, P], MDT, kind="ExternalInput").ap()
    # C pre-tiled on host: [2(n_half), HO, P(h_inner), NF]; C_imn = -C_im
    cre_d = nc.dram_tensor("C_pre_re", [2, HO, P, NF], MDT, kind="ExternalInput").ap()
    cim_d = nc.dram_tensor("C_pre_imn", [2, HO, P, NF], MDT, kind="ExternalInput").ap()
    # per-feature vectors, host-shaped [P, HO]
    lre_d = nc.dram_tensor("lam_re", [P, HO], F32, kind="ExternalInput").ap()
    lim_d = nc.dram_tensor("lam_im", [P, HO], F32, kind="ExternalInput").ap()
    limn_d = nc.dram_tensor("lam_imn", [P, HO], F32, kind="ExternalInput").ap()
    gam_d = nc.dram_tensor("gamma_v", [P, HO], F32, kind="ExternalInput").ap()
    dbc_d = nc.dram_tensor("D_bcast", [P, N], F32, kind="ExternalInput").ap()

    xre_d = nc.dram_tensor("xT_re", [H, BC], MDT, kind="ExternalOutput").ap()
    xim_d = nc.dram_tensor("xT_im", [H, BC], MDT, kind="ExternalOutput").ap()
    y_d = nc.dram_tensor("y", [BC, N], F32, kind="ExternalOutput").ap()

    MULT = mybir.AluOpType.mult
    ADD = mybir.AluOpType.add
    COPY = mybir.ActivationFunctionType.Copy

    with tile.TileContext(nc) as tc:
      for _rep in range(repeat):
        with tc.tile_pool(name="const", bufs=1) as const:
            lre = const.tile([P, HO], F32, tag="lre")
            nc.sync.dma_start(lre[:], lre_d)
            lim = const.tile([P, HO], F32, tag="lim")
            nc.sync.dma_start(lim[:], lim_d)
            limn = const.tile([P, HO], F32, tag="limn")
            nc.sync.dma_start(limn[:], limn_d)
            gam = const.tile([P, HO], F32, tag="gam")
            nc.sync.dma_start(gam[:], gam_d)
            dbc = const.tile([P, N], F32, tag="dbc")
            nc.sync.dma_start(dbc[:], dbc_d)

            with tc.tile_pool(name="xres", bufs=1) as xres:
                xre_sb = xres.tile([P, HO, BC], MDT, tag="xre")
                xim_sb = xres.tile([P, HO, BC], MDT, tag="xim")
                if mm_only:
                    nc.vector.memset(xre_sb[:].bitcast(F32), 0.0)
                    nc.vector.memset(xim_sb[:].bitcast(F32), 0.0)

                # ---------------- phase 1 ----------------
                with (
                    tc.tile_pool(name="upool", bufs=1) as upool,
                    tc.tile_pool(name="bw", bufs=2) as bw,
                    tc.tile_pool(name="st", bufs=2) as st,
                    tc.tile_pool(name="ps1", bufs=2, space="PSUM") as ps1,
                ):
                    uT_sb = upool.tile([P, KO, BC], MDT, tag="uT")
                    uT_r = uT_d.rearrange("(ko p) b -> p ko b", p=P)
                    for ko in range(KO):
                        nc.sync.dma_start(uT_sb[:, ko, :], uT_r[:, ko, :])

                    for ho in range(HO):
                        bre_t = bw.tile([P, KO, P], MDT, tag="bre")
                        nc.sync.dma_start(bre_t[:], bre_d[ho])
                        bim_t = bw.tile([P, KO, P], MDT, tag="bim")
                        nc.sync.dma_start(bim_t[:], bim_d[ho])
                        sre_t = st.tile([P, BC], F32, tag="sre")
                        nc.sync.dma_start(sre_t[:], sre_d[ho * P:(ho + 1) * P, :])
                        sim_t = st.tile([P, BC], F32, tag="sim")
                        nc.sync.dma_start(sim_t[:], sim_d[ho * P:(ho + 1) * P, :])

                        ps_u = {}
                        for comp, b_t in (("re", bre_t), ("im", bim_t)):
                            for f in range(2):
                                ps = ps1.tile([P, NF], F32, tag=f"psu_{comp}_{f}")
                                for ko in range(KO if not no_mm else 0):
                                    nc.tensor.matmul(
                                        ps[:],
                                        lhsT=b_t[:, ko, :],
                                        rhs=uT_sb[:, ko, f * NF:(f + 1) * NF],
                                        start=(ko == 0),
                                        stop=(ko == KO - 1),
                                    )
                                ps_u[(comp, f)] = ps

                        if mm_only:
                            continue
                        lre_s = lre[:, ho:ho + 1]
                        lim_s = lim[:, ho:ho + 1]
                        limn_s = limn[:, ho:ho + 1]
                        gam_s = gam[:, ho:ho + 1]

                        # x_re = s_re*l_re - s_im*l_im + gam*U_re
                        xre_o = xre_sb[:, ho, :]
                        nc.scalar.activation(xre_o, sre_t[:], COPY, scale=lre_s)
                        nc.vector.scalar_tensor_tensor(
                            xre_o, sim_t[:], limn_s, xre_o, op0=MULT, op1=ADD)
                        # x_im = s_re*l_im + s_im*l_re + gam*U_im
                        xim_o = xim_sb[:, ho, :]
                        nc.scalar.activation(xim_o, sim_t[:], COPY, scale=lre_s)
                        nc.vector.scalar_tensor_tensor(
                            xim_o, sre_t[:], lim_s, xim_o, op0=MULT, op1=ADD)
                        for f in range(2 if not no_mm else 0):
                            fs = slice(f * NF, (f + 1) * NF)
                            nc.vector.scalar_tensor_tensor(
                                xre_o[:, fs], ps_u[("re", f)][:], gam_s,
                                xre_o[:, fs], op0=MULT, op1=ADD)
                            nc.vector.scalar_tensor_tensor(
                                xim_o[:, fs], ps_u[("im", f)][:], gam_s,
                                xim_o[:, fs], op0=MULT, op1=ADD)

                # ---------------- phase 2 ----------------
                with (
                    tc.tile_pool(name="cw", bufs=3) as cw,
                    tc.tile_pool(name="ev", bufs=2) as ev,
                    tc.tile_pool(name="ps2", bufs=1, space="PSUM") as ps2,
                ):
                    for nf in range(2):
                        nfs = slice(nf * NF, (nf + 1) * NF)
                        ps_y = [ps2.tile([P, NF], F32, tag=f"psy_{bt}",
                                         name=f"psy_{nf}_{bt}")
                                for bt in range(BT)]
                        for ho in range(HO):
                            cre_t = cw.tile([P, NF], MDT, tag="cre")
                            nc.sync.dma_start(cre_t[:], cre_d[nf, ho])
                            cim_t = cw.tile([P, NF], MDT, tag="cim")
                            nc.sync.dma_start(cim_t[:], cim_d[nf, ho])
                            for bt in range(BT if not no_mm else 0):
                                bs = slice(bt * P, (bt + 1) * P)
                                nc.tensor.matmul(
                                    ps_y[bt][:],
                                    lhsT=xre_sb[:, ho, bs],
                                    rhs=cre_t[:],
                                    start=(ho == 0),
                                    stop=False,
                                )
                                nc.tensor.matmul(
                                    ps_y[bt][:],
                                    lhsT=xim_sb[:, ho, bs],
                                    rhs=cim_t[:],
                                    start=False,
                                    stop=(ho == HO - 1),
                                )
                            if nf == 0 and not mm_only:
                                # stream x out under phase-2 compute
                                nc.sync.dma_start(
                                    xre_d[ho * P:(ho + 1) * P, :], xre_sb[:, ho, :])
                                nc.sync.dma_start(
                                    xim_d[ho * P:(ho + 1) * P, :], xim_sb[:, ho, :])
                        for bt in range(BT if not mm_only else 0):
                            bs = slice(bt * P, (bt + 1) * P)
                            u_t = ev.tile([P, NF], F32, tag="ubm")
                            nc.sync.dma_start(u_t[:], ubm_d[bs, nfs])
                            du = ev.tile([P, NF], F32, tag="du")
                            nc.vector.tensor_tensor(
                                du[:], u_t[:], dbc[:, nfs], MULT)
                            y_t = ev.tile([P, NF], F32, tag="yt")
                            if no_mm:
                                nc.vector.tensor_copy(y_t[:], du[:])
                            else:
                                nc.vector.tensor_tensor(
                                    y_t[:], ps_y[bt][:], du[:], ADD)
                            nc.sync.dma_start(y_d[bs, nfs], y_t[:])

    nc.compile()
    _BUILT[key] = nc
    return nc


def kernel(inputs, state_re, state_im, B_re, B_im, C_re, C_im, D, nu, theta,
           gamma):
    nc = _build()

    f32 = np.float32
    # host-side parameter prep (fp64 for transcendentals, cast to f32)
    nu64 = nu.astype(np.float64)
    lam_mod = np.exp(-np.exp(nu64))
    lam_re = (lam_mod * np.cos(theta.astype(np.float64))).astype(f32)
    lam_im = (lam_mod * np.sin(theta.astype(np.float64))).astype(f32)

    def vec_ph(v):  # [H] -> [P, HO] with v[ho*P + p] at [p, ho]
        return np.ascontiguousarray(v.reshape(HO, P).T)

    B_pre_re = np.ascontiguousarray(
        B_re.reshape(KO, P, HO, P).transpose(2, 1, 0, 3))
    B_pre_im = np.ascontiguousarray(
        B_im.reshape(KO, P, HO, P).transpose(2, 1, 0, 3))
    C_pre_re = np.ascontiguousarray(
        C_re.reshape(HO, P, 2, NF).transpose(2, 0, 1, 3))
    C_pre_imn = np.ascontiguousarray(
        (-C_im).reshape(HO, P, 2, NF).transpose(2, 0, 1, 3))

    shared = {
        "B_pre_re": B_pre_re,
        "B_pre_im": B_pre_im,
        "C_pre_re": C_pre_re,
        "C_pre_imn": C_pre_imn,
        "lam_re": vec_ph(lam_re),
        "lam_im": vec_ph(lam_im),
        "lam_imn": vec_ph(-lam_im),
        "gamma_v": vec_ph(gamma.astype(f32)),
        "D_bcast": np.ascontiguousarray(np.broadcast_to(D.astype(f32), (P, N))),
    }

    in_maps = []
    for c in range(NCORES):
        rows = slice(c * BC, (c + 1) * BC)
        in_maps.append({
            **shared,
            "uT": np.ascontiguousarray(inputs[rows].T),
            "u_bm": np.ascontiguousarray(inputs[rows]),
            "sT_re": np.ascontiguousarray(state_re[rows].T),
            "sT_im": np.ascontiguousarray(state_im[rows].T),
        })

    res = run_bass_kernel_spmd(nc, in_maps, core_ids=list(range(NCORES)))

    y = np.empty((NCORES * BC, N), f32)
    x_re = np.empty((NCORES * BC, H), f32)
    x_im = np.empty((NCORES * BC, H), f32)
    for c in range(NCORES):
        rows = slice(c * BC, (c + 1) * BC)
        r = res.results[c]
        y[rows] = r["y"]
        x_re[rows] = r["xT_re"].T
        x_im[rows] = r["xT_im"].T
    return y, x_re, x_im


# revision 8
# speedup vs baseline: 334.1577x; 2.7758x over previous
"""LinearRecurrentUnitCell Bass/Tile kernel for 8 Trainium2 NeuronCores.

Computation (per reference):
    lam = exp(-exp(nu)) * exp(i*theta)                  # [H] complex
    U   = gamma * (inputs @ (B_re + i B_im))            # [B,H] complex
    x'  = (state_re + i state_im) * lam + U             # [B,H] complex
    y   = Re(x' @ (C_re + i C_im)) + D * inputs         # [B,N]
    returns (y, Re(x'), Im(x'))

Sharding: data-parallel over batch (8192 -> 8 x 1024). Parameters replicated.

Device layout strategy ("feature-major"): all activations are kept
transposed (feature dim on partitions) so every matmul uses operands
exactly as stored -- no on-device transposes:
  phase 1:  UT[h,b]  = sum_n B[n,h] * uT[n,b]      (lhsT=B tile, rhs=uT)
  elemwise: xT = lam (.) sT + gamma (.) UT          (per-partition scalars)
  phase 2:  y[b,n]   = sum_h xT[h,b] * C[h,n]       (lhsT=xT tile, rhs=C tile)
Host transposes inputs/states in and xT out (numpy; not on-HW time).
"""

import os

import numpy as np

import concourse.bass as bass
import concourse.tile as tile
from concourse import bacc, mybir
from concourse.bass_utils import run_bass_kernel_spmd

P = 128            # partitions
N = 1024           # n_rec
H = 2048           # d_hidden
BC = 1024          # batch per core
NCORES = 8
HO = H // P        # 16 h-tiles
KO = N // P        # 8 k-tiles (phase 1 contraction)
BT = BC // P       # 8 batch tiles (phase 2 output partition)
NF = 512           # matmul moving free dim (fp32 max)

F32 = mybir.dt.float32
F32R = mybir.dt.float32r

# matmul dtype: "f32r" = relaxed-precision fp32 at full PE rate,
# "f32" = exact fp32 at 1/4 PE rate.
MM_MODE = os.environ.get("LRU_MM_DT", "f32r")
MDT = F32R if MM_MODE == "f32r" else F32


_BUILT = {}


def _build(repeat=1, variant="full"):
    key = (repeat, variant)
    if key in _BUILT:
        return _BUILT[key]
    mm_only = variant in ("mm_only", "mm_dense")
    no_mm = variant == "no_mm"
    mm_rep = 2 if variant == "mm_dense" else 1

    nc = bacc.Bacc("TRN2", target_bir_lowering=False, debug=False)

    # --- DRAM I/O (per-core shapes) ---
    uT_d = nc.dram_tensor("uT", [N, BC], MDT, kind="ExternalInput").ap()
    ubm_d = nc.dram_tensor("u_bm", [BC, N], F32, kind="ExternalInput").ap()
    sre_d = nc.dram_tensor("sT_re", [H, BC], F32, kind="ExternalInput").ap()
    sim_d = nc.dram_tensor("sT_im", [H, BC], F32, kind="ExternalInput").ap()
    # B pre-tiled on host: [HO, P(n_inner), KO, P(h_inner)]
    bre_d = nc.dram_tensor("B_pre_re", [HO, P, KO, P], MDT, kind="ExternalInput").ap()
    bim_d = nc.dram_tensor("B_pre_im", [HO, P, KO, P], MDT, kind="ExternalInput").ap()
    # C pre-tiled on host: [2(n_half), HO, P(h_inner), NF]; C_imn = -C_im
    cre_d = nc.dram_tensor("C_pre_re", [2, HO, P, NF], MDT, kind="ExternalInput").ap()
    cim_d = nc.dram_tensor("C_pre_imn", [2, HO, P, NF], MDT, kind="ExternalInput").ap()
    # per-feature vectors, host-shaped [P, HO]
    lre_d = nc.dram_tensor("lam_re", [P, HO], F32, kind="ExternalInput").ap()
    lim_d = nc.dram_tensor("lam_im", [P, HO], F32, kind="ExternalInput").ap()
    limn_d = nc.dram_tensor("lam_imn", [P, HO], F32, kind="ExternalInput").ap()
    gam_d = nc.dram_tensor("gamma_v", [P, HO], F32, kind="ExternalInput").ap()
    dbc_d = nc.dram_tensor("D_bcast", [P, N], F32, kind="ExternalInput").ap()

    xre_d = nc.dram_tensor("xT_re", [H, BC], MDT, kind="ExternalOutput").ap()
    xim_d = nc.dram_tensor("xT_im", [H, BC], MDT, kind="ExternalOutput").ap()
    y_d = nc.dram_tensor("y", [BC, N], F32, kind="ExternalOutput").ap()

    MULT = mybir.AluOpType.mult
    ADD = mybir.AluOpType.add
    COPY = mybir.ActivationFunctionType.Copy

    with tile.TileContext(nc) as tc:
      for _rep in range(repeat):
        with tc.tile_pool(name="const", bufs=1) as const:
            lre = const.tile([P, HO], F32, tag="lre")
            nc.sync.dma_start(lre[:], lre_d)
            lim = const.tile([P, HO], F32, tag="lim")
            nc.sync.dma_start(lim[:], lim_d)
            limn = const.tile([P, HO], F32, tag="limn")
            nc.sync.dma_start(limn[:], limn_d)
            gam = const.tile([P, HO], F32, tag="gam")
            nc.sync.dma_start(gam[:], gam_d)
            dbc = const.tile([P, N], F32, tag="dbc")
            nc.sync.dma_start(dbc[:], dbc_d)

            with tc.tile_pool(name="xres", bufs=1) as xres:
                xre_sb = xres.tile([P, HO, BC], MDT, tag="xre")
                xim_sb = xres.tile([P, HO, BC], MDT, tag="xim")
                if mm_only:
                    nc.vector.memset(xre_sb[:].bitcast(F32), 0.0)
                    nc.vector.memset(xim_sb[:].bitcast(F32), 0.0)

                # ---------------- phase 1 ----------------
                with (
                    tc.tile_pool(name="upool", bufs=1) as upool,
                    tc.tile_pool(name="bw", bufs=2) as bw,
                    tc.tile_pool(name="st", bufs=2) as st,
                    tc.tile_pool(name="ps1", bufs=2, space="PSUM") as ps1,
                ):
                    uT_sb = upool.tile([P, KO, BC], MDT, tag="uT")
                    uT_r = uT_d.rearrange("(ko p) b -> p ko b", p=P)
                    for ko in range(KO):
                        nc.sync.dma_start(uT_sb[:, ko, :], uT_r[:, ko, :])

                    for ho in range(HO):
                        bre_t = bw.tile([P, KO, P], MDT, tag="bre")
                        nc.sync.dma_start(bre_t[:], bre_d[ho])
                        bim_t = bw.tile([P, KO, P], MDT, tag="bim")
                        nc.sync.dma_start(bim_t[:], bim_d[ho])
                        sre_t = st.tile([P, BC], F32, tag="sre")
                        nc.sync.dma_start(sre_t[:], sre_d[ho * P:(ho + 1) * P, :])
                        sim_t = st.tile([P, BC], F32, tag="sim")
                        nc.sync.dma_start(sim_t[:], sim_d[ho * P:(ho + 1) * P, :])

                        ps_u = {}
                        for comp, b_t in (("re", bre_t), ("im", bim_t)):
                            for f in range(2):
                                ps = ps1.tile([P, NF], F32, tag=f"psu_{comp}_{f}")
                                for ko in range(KO * mm_rep if not no_mm else 0):
                                    nc.tensor.matmul(
                                        ps[:],
                                        lhsT=b_t[:, ko % KO, :],
                                        rhs=uT_sb[:, ko % KO, f * NF:(f + 1) * NF],
                                        start=(ko == 0),
                                        stop=(ko == KO * mm_rep - 1),
                                    )
                                ps_u[(comp, f)] = ps

                        if mm_only:
                            continue
                        lre_s = lre[:, ho:ho + 1]
                        lim_s = lim[:, ho:ho + 1]
                        limn_s = limn[:, ho:ho + 1]
                        gam_s = gam[:, ho:ho + 1]

                        # x_re = s_re*l_re - s_im*l_im + gam*U_re
                        xre_o = xre_sb[:, ho, :]
                        nc.scalar.activation(xre_o, sre_t[:], COPY, scale=lre_s)
                        nc.vector.scalar_tensor_tensor(
                            xre_o, sim_t[:], limn_s, xre_o, op0=MULT, op1=ADD)
                        # x_im = s_re*l_im + s_im*l_re + gam*U_im
                        xim_o = xim_sb[:, ho, :]
                        nc.scalar.activation(xim_o, sim_t[:], COPY, scale=lre_s)
                        nc.vector.scalar_tensor_tensor(
                            xim_o, sre_t[:], lim_s, xim_o, op0=MULT, op1=ADD)
                        for f in range(2 if not no_mm else 0):
                            fs = slice(f * NF, (f + 1) * NF)
                            nc.vector.scalar_tensor_tensor(
                                xre_o[:, fs], ps_u[("re", f)][:], gam_s,
                                xre_o[:, fs], op0=MULT, op1=ADD)
                            nc.vector.scalar_tensor_tensor(
                                xim_o[:, fs], ps_u[("im", f)][:], gam_s,
                                xim_o[:, fs], op0=MULT, op1=ADD)

                # ---------------- phase 2 ----------------
                with (
                    tc.tile_pool(name="cw", bufs=3) as cw,
                    tc.tile_pool(name="ev", bufs=2) as ev,
                    tc.tile_pool(name="ps2", bufs=1, space="PSUM") as ps2,
                ):
                    for nf in range(2):
                        nfs = slice(nf * NF, (nf + 1) * NF)
                        ps_y = [ps2.tile([P, NF], F32, tag=f"psy_{bt}",
                                         name=f"psy_{nf}_{bt}")
                                for bt in range(BT)]
                        for ho in range(HO):
                            cre_t = cw.tile([P, NF], MDT, tag="cre")
                            nc.sync.dma_start(cre_t[:], cre_d[nf, ho])
                            cim_t = cw.tile([P, NF], MDT, tag="cim")
                            nc.sync.dma_start(cim_t[:], cim_d[nf, ho])
                            for bt in range(BT if not no_mm else 0):
                                bs = slice(bt * P, (bt + 1) * P)
                                for _mr in range(mm_rep):
                                    nc.tensor.matmul(
                                        ps_y[bt][:],
                                        lhsT=xre_sb[:, ho, bs],
                                        rhs=cre_t[:],
                                        start=(ho == 0 and _mr == 0),
                                        stop=False,
                                    )
                                    nc.tensor.matmul(
                                        ps_y[bt][:],
                                        lhsT=xim_sb[:, ho, bs],
                                        rhs=cim_t[:],
                                        start=False,
                                        stop=(ho == HO - 1 and _mr == mm_rep - 1),
                                    )
                            if nf == 0 and not mm_only:
                                # stream x out under phase-2 compute
                                nc.sync.dma_start(
                                    xre_d[ho * P:(ho + 1) * P, :], xre_sb[:, ho, :])
                                nc.sync.dma_start(
                                    xim_d[ho * P:(ho + 1) * P, :], xim_sb[:, ho, :])
                        for bt in range(BT if not mm_only else 0):
                            bs = slice(bt * P, (bt + 1) * P)
                            u_t = ev.tile([P, NF], F32, tag="ubm")
                            nc.sync.dma_start(u_t[:], ubm_d[bs, nfs])
                            du = ev.tile([P, NF], F32, tag="du")
                            nc.vector.tensor_tensor(
                                du[:], u_t[:], dbc[:, nfs], MULT)
                            y_t = ev.tile([P, NF], F32, tag="yt")
                            if no_mm:
                                nc.vector.tensor_copy(y_t[:], du[:])
                            else:
                                nc.vector.tensor_tensor(
                                    y_t[:], ps_y[bt][:], du[:], ADD)
                            nc.sync.dma_start(y_d[bs, nfs], y_t[:])

    nc.compile()
    _BUILT[key] = nc
    return nc


def kernel(inputs, state_re, state_im, B_re, B_im, C_re, C_im, D, nu, theta,
           gamma):
    nc = _build()

    f32 = np.float32
    # host-side parameter prep (fp64 for transcendentals, cast to f32)
    nu64 = nu.astype(np.float64)
    lam_mod = np.exp(-np.exp(nu64))
    lam_re = (lam_mod * np.cos(theta.astype(np.float64))).astype(f32)
    lam_im = (lam_mod * np.sin(theta.astype(np.float64))).astype(f32)

    def vec_ph(v):  # [H] -> [P, HO] with v[ho*P + p] at [p, ho]
        return np.ascontiguousarray(v.reshape(HO, P).T)

    B_pre_re = np.ascontiguousarray(
        B_re.reshape(KO, P, HO, P).transpose(2, 1, 0, 3))
    B_pre_im = np.ascontiguousarray(
        B_im.reshape(KO, P, HO, P).transpose(2, 1, 0, 3))
    C_pre_re = np.ascontiguousarray(
        C_re.reshape(HO, P, 2, NF).transpose(2, 0, 1, 3))
    C_pre_imn = np.ascontiguousarray(
        (-C_im).reshape(HO, P, 2, NF).transpose(2, 0, 1, 3))

    shared = {
        "B_pre_re": B_pre_re,
        "B_pre_im": B_pre_im,
        "C_pre_re": C_pre_re,
        "C_pre_imn": C_pre_imn,
        "lam_re": vec_ph(lam_re),
        "lam_im": vec_ph(lam_im),
        "lam_imn": vec_ph(-lam_im),
        "gamma_v": vec_ph(gamma.astype(f32)),
        "D_bcast": np.ascontiguousarray(np.broadcast_to(D.astype(f32), (P, N))),
    }

    in_maps = []
    for c in range(NCORES):
        rows = slice(c * BC, (c + 1) * BC)
        in_maps.append({
            **shared,
            "uT": np.ascontiguousarray(inputs[rows].T),
            "u_bm": np.ascontiguousarray(inputs[rows]),
            "sT_re": np.ascontiguousarray(state_re[rows].T),
            "sT_im": np.ascontiguousarray(state_im[rows].T),
        })

    res = run_bass_kernel_spmd(nc, in_maps, core_ids=list(range(NCORES)))

    y = np.empty((NCORES * BC, N), f32)
    x_re = np.empty((NCORES * BC, H), f32)
    x_im = np.empty((NCORES * BC, H), f32)
    for c in range(NCORES):
        rows = slice(c * BC, (c + 1) * BC)
        r = res.results[c]
        y[rows] = r["y"]
        x_re[rows] = r["xT_re"].T
        x_im[rows] = r["xT_im"].T
    return y, x_re, x_im
